# revision 8
# baseline (speedup 1.0000x reference)
"""NonLocalBlock2D (embedded-gaussian non-local attention) on 8 TRN2 NeuronCores.

v4 — balance ACT/DVE around the exp floor, strip all removable work off the
two exp engines, and shorten startup/drain. Sharding as v3: core k handles
sample b=k//2, query rows h*3200:(h+1)*3200 (h=k%2); keys are the full 6400
positions (x rotated per-core so this core's queries are cols 0:3200).

Cost-model structure (TimelineSim):
  - f matmul: 1 cyc/row f32r (>=256 wide); y runs transposed (e stationary,
    gT moving, 33 rows per 128q x 128k tile) as in v3.
  - exp is split ACT (real Exp, 0.833ns/el) : DVE (Schraudolph int16 bits,
    1.04ns/el) by a time-weighted greedy assignment per block that accounts
    for each engine's fixed per-block work.
  - g conv runs in bf16 with a host-sent ones row (xb65[64]=1, wgb65 row 64
    = b_g, col 32 = denominator ones) so its drain is a pure copy, placeable
    on ACT (activation Copy shares the Exp act table - no table reload) or
    DVE. fp32r matmuls are restricted to K<=64 by walrus, so the f/theta
    convs stay K=64 f32r and the theta bias rides the DVE drain.
  - residual (+x) folded into the out-conv psum group via an identity-matmul
    (lhsT=I64, rhs=xr) so the output drain is also a pure copy on ACT.
  - f32r operands come straight from DMA (f32r dram tensors); no on-device
    rounding copies at startup.
  - QBLOCKS [5x512, 384, 256]: all widths multiples of 128 (no overlapped
    y subs) and >=256 (f32r 1cyc); small last block shortens the drain.
  - epilogue steps are spread one-per-pair-slot to avoid DVE bursts.
  - PSUM: fps ring 3 x [128,2,512] (6 banks) + py ring 2 x [128,512] = 8;
    z conv targets the dead py bank; next-block theta lands there too.
"""

import numpy as np
import ml_dtypes

import concourse.bass as bass
import concourse.tile as tile
from concourse import bacc
from concourse import mybir
from concourse.bass_utils import run_bass_kernel_spmd

B, C, HH, WW = 4, 64, 80, 80
N = HH * WW            # 6400 keys per sample
NQ = N // 2            # 3200 queries per core
INTER = 32
NCORES = 8
MC = 128               # keys per chunk
NMC = N // MC          # 50
NPAIR = NMC // 2       # 25 f/exp pairs per query block
GT_W = INTER + 1       # 32 g-channels + ones column (denominator)

F32 = mybir.dt.float32
F32R = mybir.dt.float32r
BF16 = mybir.dt.bfloat16
I16 = mybir.dt.int16
EXP = mybir.ActivationFunctionType.Exp
COPY = mybir.ActivationFunctionType.Copy
ADD = mybir.AluOpType.add
MULT = mybir.AluOpType.mult

BN_EPS = 1e-4

# Schraudolph fast-exp constants (bf16 bit pattern as int16)
A_EXP = 184.6649652337873   # 2^7 * log2(e)
B_EXP = 16250.5             # 2^7 * (127 - 0.0430)

QBLOCKS = [(0, 512), (512, 512), (1024, 512), (1536, 512), (2048, 512),
           (2560, 384), (2944, 256)]

XSL = [(i * 512, 512) for i in range(12)] + [(6144, 256)]  # x DMA chunks


def _exp_costs(w):
    # engine-busy ns for one [128,2,w] exp pair (incl. non-pipelineable init)
    return (2 * w * 0.8333 + 185.0, 2 * w * 1.0417 + 125.0)


def _mk_engs(npair, w, base_a, base_d):
    """True = ACT. Greedy time-balanced assignment given fixed extras."""
    ca, cd = _exp_costs(w)
    ta, td = base_a, base_d
    engs = []
    for _ in range(npair):
        if ta + ca <= td + cd:
            engs.append(True)
            ta += ca
        else:
            engs.append(False)
            td += cd
    return engs


def _emit(tc, d):
    nc = tc.nc

    with tc.tile_pool(name="singles", bufs=1) as singles:
        xfr = singles.tile([C, N], F32R, tag="xfr")
        xb65 = singles.tile([C + 1, N], BF16, tag="xb65")
        wm = singles.tile([C, C], F32R, tag="wm")
        btau = singles.tile([C, 1], F32, tag="btau")
        wgb = singles.tile([C + 1, GT_W], BF16, tag="wgb")
        womt = singles.tile([128, 4, C], BF16, tag="wom")
        i64 = singles.tile([C, C], F32R, tag="i64")
        xrr = singles.tile([C, NQ], F32R, tag="xrr")
        th = singles.tile([C, NQ], F32R, tag="th")
        gt = singles.tile([128, NMC, GT_W], BF16, tag="gt")

        # ---- DMAs. HWDGE (SP) serializes dispatches at ~625ns; put the
        # first-needed things first and push a few x chunks through the Pool
        # SWDGE queue so the x stream outruns the f consumption.
        def xdma(i, eng=nc.sync):
            off, xw = XSL[i]
            eng.dma_start(xfr[:, off : off + xw], d["xf"][:, off : off + xw])

        def xbdma(i, eng=nc.sync):
            off = i * 1600
            eng.dma_start(xb65[:, off : off + 1600],
                          d["xb"][:, off : off + 1600])

        xdma(0)
        nc.sync.dma_start(wm[:], d["wm"][:])
        nc.sync.dma_start(btau[:], d["btau"][:])
        nc.sync.dma_start(wgb[:], d["wg"][:])
        xbdma(0, nc.gpsimd)
        xdma(1)
        xdma(2)
        xdma(3, nc.gpsimd)
        xdma(4)
        xbdma(1, nc.gpsimd)
        xdma(5)
        xdma(6)
        xbdma(2, nc.gpsimd)
        xdma(7)
        xdma(8)
        xbdma(3, nc.gpsimd)
        xdma(9)
        xdma(10)
        xdma(11, nc.gpsimd)
        xdma(12)
        nc.sync.dma_start(
            womt[:].rearrange("p a b -> p (a b)"), d["wom"][:]
        )
        nc.sync.dma_start(i64[:], d["i64"][:])
        for i in range(2):
            sl = slice(i * 1600, (i + 1) * 1600)
            nc.sync.dma_start(xrr[:, sl], d["xr"][:, sl])

        with tc.tile_pool(name="fps", bufs=3, space="PSUM") as fps, \
             tc.tile_pool(name="yps", bufs=2, space="PSUM") as yps, \
             tc.tile_pool(name="esb", bufs=12) as esb, \
             tc.tile_pool(name="ep", bufs=4) as ep:

            def gt_batch(c0, nb, use_act):
                pgt = fps.tile([128, 2, 512], F32, tag="f")
                pg = pgt[:, 0, : nb * GT_W].rearrange(
                    "p (a b) -> p a b", b=GT_W)
                for i in range(nb):
                    nc.tensor.matmul(
                        pg[:, i, :],
                        lhsT=xb65[:, (c0 + i) * MC : (c0 + i + 1) * MC],
                        rhs=wgb[:],
                        start=(i == 0), stop=(i == nb - 1),
                    )
                dst = gt[:, c0 : c0 + nb, :]
                if use_act:
                    nc.scalar.activation(dst, pg[:, :nb, :], COPY)
                else:
                    nc.vector.tensor_copy(dst, pg[:, :nb, :])

            def th_slice(q0, w, psrc=None):
                if psrc is None:
                    pt = fps.tile([128, 2, 512], F32, tag="f")
                    pp = pt[0:C, 0, :]
                else:
                    pp = psrc
                nc.tensor.matmul(
                    pp[:, :w], lhsT=wm[:], rhs=xfr[:, q0 : q0 + w],
                    start=True, stop=True,
                )
                nc.vector.tensor_scalar_add(
                    th[:, q0 : q0 + w], pp[:, :w], btau[:])

            def make_epi_steps(pyf, py, subs, q0, w, fin=False):
                """List of closures, executed one per pair slot."""
                nsub = len(subs)
                holder = {}

                def s_pys():
                    pys = ep.tile([128, 4, GT_W], F32, tag="pys")
                    nc.scalar.activation(
                        pys[:, :nsub, :], py[:, :nsub, :], COPY)
                    holder["pys"] = pys

                def s_recip():
                    pys = holder["pys"]
                    r = ep.tile([128, 4], F32, tag="r")
                    scr = ep.tile([128, 4], F32, tag="scr")
                    dview = pys[:, :, INTER : INTER + 1].rearrange(
                        "p a o -> p (a o)")
                    nc.vector.reciprocal_approx_accurate(
                        r[:, :nsub], dview[:, :nsub], scr[:, :nsub])
                    holder["r"] = r

                z = pyf[0:C, :]
                cnt = [0]
                ncv = 4 * nsub   # strip matmuls; residual matmul closes

                def mk_sub(si, soff):
                    def s_sub():
                        pys = holder["pys"]
                        r = holder["r"]
                        ynt = ep.tile([128, INTER], BF16, tag="ynt")
                        nc.gpsimd.tensor_scalar(
                            ynt[:], pys[:, si, :INTER], r[:, si : si + 1],
                            None, op0=MULT,
                        )
                        bt = ep.tile([128, INTER], BF16, tag="bt")
                        nc.vector.transpose(bt[:], ynt[:])
                        for i in range(4):
                            bp = 32 * i
                            nc.tensor.matmul(
                                z[:, soff + bp : soff + bp + 32],
                                lhsT=womt[:, i, :],
                                rhs=bt[:],
                                start=(cnt[0] == 0),
                                stop=False,
                            )
                            cnt[0] += 1
                    return s_sub

                def s_out():
                    nc.tensor.matmul(
                        z[:, :w], lhsT=i64[:], rhs=xrr[:, q0 : q0 + w],
                        start=False, stop=True,
                    )
                    o = ep.tile([C, 512], F32, tag="o")
                    nc.scalar.activation(o[:, :w], z[:, :w], COPY)
                    nc.sync.dma_start(d["out"][:, q0 : q0 + w], o[:, :w])

                steps = [s_pys, s_recip]
                steps += [mk_sub(si, soff) for si, soff in enumerate(subs)]
                steps.append(s_out)
                return steps

            # per-block fixed engine extras (ns) for the greedy exp split
            def extras(qi, w, nsub):
                ca_o = w * 0.8333 + 143.0          # o copy on ACT
                ca_pys = nsub * GT_W * 0.8333 + 143.0
                base_a = ca_o + ca_pys
                base_d = 65.0 + nsub * 94.0        # recip + bt transposes
                if qi + 1 < len(QBLOCKS):
                    base_d += QBLOCKS[qi + 1][1] * 1.0417 + 125.0  # th drain
                if qi == 0:
                    base_a += 4 * 405.0            # gt drains on ACT
                    base_d += 3 * 400.0 + 533.0 + 125.0   # gt + th0
                return base_a, base_d

            prev_steps = []
            prev_pyf = None
            for qi, (q0, w) in enumerate(QBLOCKS):
                if qi == 0:
                    th_slice(q0, w)
                subs = list(range(0, w, 128))
                nsub = len(subs)
                pyf = yps.tile([128, 512], F32, tag="py")
                py = pyf[:, : 4 * GT_W].rearrange("p (a b) -> p a b", b=GT_W)
                pending = []

                def flush_y(py=py, subs=subs, pending=pending, nsub=nsub):
                    e, c0, gn = pending.pop(0)
                    for j in range(gn):
                        c = c0 + j
                        for si, soff in enumerate(subs):
                            nc.tensor.matmul(
                                py[:, si, :],
                                lhsT=e[:, j, soff : soff + 128],
                                rhs=gt[:, c, :],
                                start=(c == 0 and si == 0),
                                stop=(c == NMC - 1 and si == nsub - 1),
                            )

                base_a, base_d = extras(qi, w, nsub)
                engs = _mk_engs(NPAIR, w, base_a, base_d)
                gtd = [True, False, True, False, True, False, True]

                if qi == 0:
                    gt_batch(0, 8, gtd[0])
                for p in range(NPAIR):
                    pf = fps.tile([128, 2, 512], F32, tag="f")
                    for j in range(2):
                        c = 2 * p + j
                        nc.tensor.matmul(
                            pf[:, j, :w],
                            lhsT=xfr[:, c * MC : (c + 1) * MC],
                            rhs=th[:, q0 : q0 + w],
                            start=True, stop=True,
                        )
                    if qi == 0 and p % 4 == 1 and p < 22:
                        k = (p - 1) // 4
                        c0 = 8 * k + 8
                        gt_batch(c0, min(8, NMC - c0), gtd[k + 1])
                    if p >= 1 and prev_steps:
                        prev_steps.pop(0)()
                        if p >= 3 and prev_steps:  # catch up 2/slot late
                            prev_steps.pop(0)()
                    if p == 20 and qi + 1 < len(QBLOCKS):
                        nq0, nw = QBLOCKS[qi + 1]
                        th_slice(nq0, nw,
                                 psrc=(prev_pyf[0:C, :nw]
                                       if prev_pyf is not None else None))
                    lag = 1 if qi == len(QBLOCKS) - 1 else 3
                    if len(pending) > lag:
                        flush_y()
                    e = esb.tile([128, 2, 512], BF16, tag="e")
                    if engs[p]:
                        nc.scalar.activation(e[:, :, :w], pf[:, :, :w], EXP)
                    else:
                        nc.vector.tensor_scalar(
                            e[:, :, :w].bitcast(I16), pf[:, :, :w],
                            A_EXP, B_EXP, op0=MULT, op1=ADD,
                        )
                    pending.append((e, 2 * p, 2))
                while pending:
                    flush_y()
                prev_pyf = pyf
                prev_steps = make_epi_steps(
                    pyf, py, subs, q0, w, fin=(q0 + w >= NQ))
            for s in prev_steps:
                s()


def build():
    nc = bacc.Bacc("TRN2", target_bir_lowering=False, debug=False)
    d = {}
    d["xf"] = nc.dram_tensor("xf", [C, N], F32R, kind="ExternalInput").ap()
    d["xb"] = nc.dram_tensor("xb", [C + 1, N], BF16, kind="ExternalInput").ap()
    d["xr"] = nc.dram_tensor("xr", [C, NQ], F32R, kind="ExternalInput").ap()
    d["wm"] = nc.dram_tensor("wm", [C, C], F32R, kind="ExternalInput").ap()
    d["btau"] = nc.dram_tensor("btau", [C, 1], F32, kind="ExternalInput").ap()
    d["wg"] = nc.dram_tensor("wg", [C + 1, GT_W], BF16,
                             kind="ExternalInput").ap()
    d["wom"] = nc.dram_tensor("wom", [128, 4 * C], BF16,
                              kind="ExternalInput").ap()
    d["i64"] = nc.dram_tensor("i64", [C, C], F32R, kind="ExternalInput").ap()
    d["out"] = nc.dram_tensor("out", [C, NQ], F32, kind="ExternalOutput").ap()
    with tile.TileContext(nc) as tc:
        _emit(tc, d)
    nc.compile()
    return nc


def make_in_maps(x, w_theta, b_theta, w_phi, b_phi, w_g, b_g,
                 w_out, b_out, bn_gamma, bn_beta, bn_mean, bn_var):
    x = np.ascontiguousarray(np.asarray(x, dtype=np.float32))
    w_theta = np.asarray(w_theta, np.float32)
    b_theta = np.asarray(b_theta, np.float32)
    w_phi = np.asarray(w_phi, np.float32)
    w_g = np.asarray(w_g, np.float32)
    b_g = np.asarray(b_g, np.float32)
    w_out = np.asarray(w_out, np.float32)
    b_out = np.asarray(b_out, np.float32)
    bn_gamma = np.asarray(bn_gamma, np.float32)
    bn_beta = np.asarray(bn_beta, np.float32)
    bn_mean = np.asarray(bn_mean, np.float32)
    bn_var = np.asarray(bn_var, np.float32)

    inv = bn_gamma / np.sqrt(bn_var + BN_EPS)
    wo_folded = w_out * inv[:, None]                       # [64,32]
    bo_folded = (b_out - bn_mean) * inv + bn_beta          # [64]

    # f[q,k] = x_q.T M x_k + (Wph.T b_theta).x_q-side bias carried by theta;
    # per-query terms are softmax-row-invariant and dropped.
    wm_l = np.ascontiguousarray(w_theta.T @ w_phi)         # [64,64]
    btau_c = np.ascontiguousarray((w_phi.T @ b_theta)[:, None])  # [64,1]
    wg65 = np.zeros((C + 1, GT_W), np.float32)
    wg65[:C, :INTER] = w_g.T
    wg65[C, :INTER] = b_g
    wg65[C, INTER] = 1.0                                   # denominator ones
    wg65 = np.ascontiguousarray(wg65.astype(ml_dtypes.bfloat16))
    wom = np.zeros((128, 4, C), np.float32)
    for i in range(4):
        wom[32 * i : 32 * i + 32, i, :] = wo_folded.T
    wom = np.ascontiguousarray(
        wom.reshape(128, 4 * C).astype(ml_dtypes.bfloat16))
    i64 = np.eye(C, dtype=np.float32)

    xflat = x.reshape(B, C, N)
    in_maps = []
    for core in range(NCORES):
        b, h = divmod(core, 2)
        xrot = np.ascontiguousarray(np.roll(xflat[b], -h * NQ, axis=1))
        xb65 = np.ones((C + 1, N), np.float32)
        xb65[:C] = xrot
        xb65 = np.ascontiguousarray(xb65.astype(ml_dtypes.bfloat16))
        xres = np.ascontiguousarray(xrot[:, :NQ] + bo_folded[:, None])
        in_maps.append(
            {
                "xf": xrot,
                "xb": xb65,
                "xr": xres,
                "wm": wm_l,
                "btau": btau_c,
                "wg": wg65,
                "wom": wom,
                "i64": i64,
            }
        )
    return in_maps


def assemble_out(results):
    out = np.empty((B, C, N), np.float32)
    for core in range(NCORES):
        b, h = divmod(core, 2)
        out[b][:, h * NQ : (h + 1) * NQ] = results[core]["out"]
    return out.reshape(B, C, HH, WW)


_NC_CACHE = [None]


def kernel(**inputs):
    if _NC_CACHE[0] is None:
        _NC_CACHE[0] = build()
    nc = _NC_CACHE[0]
    in_maps = make_in_maps(**inputs)
    res = run_bass_kernel_spmd(nc, in_maps, core_ids=list(range(NCORES)))
    return assemble_out(res.results)


# revision 14
# speedup vs baseline: 1.0007x; 1.0007x over previous
"""NonLocalBlock2D (embedded-gaussian non-local attention) on 8 TRN2 NeuronCores.

v4 — balance ACT/DVE around the exp floor, strip all removable work off the
two exp engines, and shorten startup/drain. Sharding as v3: core k handles
sample b=k//2, query rows h*3200:(h+1)*3200 (h=k%2); keys are the full 6400
positions (x rotated per-core so this core's queries are cols 0:3200).

Cost-model structure (TimelineSim):
  - f matmul: 1 cyc/row f32r (>=256 wide); y runs transposed (e stationary,
    gT moving, 33 rows per 128q x 128k tile) as in v3.
  - exp is split ACT (real Exp, 0.833ns/el) : DVE (Schraudolph int16 bits,
    1.04ns/el) by a time-weighted greedy assignment per block that accounts
    for each engine's fixed per-block work.
  - g conv runs in bf16 with a host-sent ones row (xb65[64]=1, wgb65 row 64
    = b_g, col 32 = denominator ones) so its drain is a pure copy, placeable
    on ACT (activation Copy shares the Exp act table - no table reload) or
    DVE. fp32r matmuls are restricted to K<=64 by walrus, so the f/theta
    convs stay K=64 f32r and the theta bias rides the DVE drain.
  - residual (+x) folded into the out-conv psum group via an identity-matmul
    (lhsT=I64, rhs=xr) so the output drain is also a pure copy on ACT.
  - f32r operands come straight from DMA (f32r dram tensors); no on-device
    rounding copies at startup.
  - QBLOCKS [5x512, 384, 256]: all widths multiples of 128 (no overlapped
    y subs) and >=256 (f32r 1cyc); small last block shortens the drain.
  - epilogue steps are spread one-per-pair-slot to avoid DVE bursts.
  - PSUM: fps ring 3 x [128,2,512] (6 banks) + py ring 2 x [128,512] = 8;
    z conv targets the dead py bank; next-block theta lands there too.
"""

import numpy as np
import ml_dtypes

import concourse.bass as bass
import concourse.tile as tile
from concourse import bacc
from concourse import mybir
from concourse.bass_utils import run_bass_kernel_spmd

B, C, HH, WW = 4, 64, 80, 80
N = HH * WW            # 6400 keys per sample
NQ = N // 2            # 3200 queries per core
INTER = 32
NCORES = 8
MC = 128               # keys per chunk
NMC = N // MC          # 50
NPAIR = NMC // 2       # 25 f/exp pairs per query block
GT_W = INTER + 1       # 32 g-channels + ones column (denominator)

F32 = mybir.dt.float32
F32R = mybir.dt.float32r
BF16 = mybir.dt.bfloat16
I16 = mybir.dt.int16
EXP = mybir.ActivationFunctionType.Exp
COPY = mybir.ActivationFunctionType.Copy
ADD = mybir.AluOpType.add
MULT = mybir.AluOpType.mult

BN_EPS = 1e-4

# Schraudolph fast-exp constants (bf16 bit pattern as int16)
A_EXP = 184.6649652337873   # 2^7 * log2(e)
B_EXP = 16250.5             # 2^7 * (127 - 0.0430)

QBLOCKS = [(0, 512), (512, 512), (1024, 512), (1536, 512), (2048, 512),
           (2560, 384), (2944, 256)]

XSL = [(i * 512, 512) for i in range(12)] + [(6144, 256)]  # x DMA chunks


def _exp_costs(w):
    # engine-busy ns for one [128,2,w] exp pair (incl. non-pipelineable init)
    return (2 * w * 0.8333 + 185.0, 2 * w * 1.0417 + 125.0)


def _mk_engs(npair, w, base_a, base_d):
    """True = ACT. Greedy time-balanced assignment given fixed extras."""
    ca, cd = _exp_costs(w)
    ta, td = base_a, base_d
    engs = []
    for _ in range(npair):
        if ta + ca <= td + cd:
            engs.append(True)
            ta += ca
        else:
            engs.append(False)
            td += cd
    return engs


def _emit(tc, d):
    nc = tc.nc

    with tc.tile_pool(name="singles", bufs=1) as singles:
        xfr = singles.tile([C, N], F32R, tag="xfr")
        xb65 = singles.tile([C + 1, N], BF16, tag="xb65")
        wm = singles.tile([C, C], F32R, tag="wm")
        btau = singles.tile([C, 1], F32, tag="btau")
        wgb = singles.tile([C + 1, GT_W], BF16, tag="wgb")
        womt = singles.tile([128, 4, C], BF16, tag="wom")
        i64 = singles.tile([C, C], F32R, tag="i64")
        xrr = singles.tile([C, NQ], F32R, tag="xrr")
        th = singles.tile([C, NQ], F32R, tag="th")
        gt = singles.tile([128, NMC, GT_W], BF16, tag="gt")

        # ---- DMAs. HWDGE (SP) serializes dispatches at ~625ns; put the
        # first-needed things first and push a few x chunks through the Pool
        # SWDGE queue so the x stream outruns the f consumption.
        def xdma(i, eng=nc.sync):
            off, xw = XSL[i]
            eng.dma_start(xfr[:, off : off + xw], d["xf"][:, off : off + xw])

        def xbdma(i, eng=nc.sync):
            off = i * 1600
            eng.dma_start(xb65[:, off : off + 1600],
                          d["xb"][:, off : off + 1600])

        xdma(0)
        nc.sync.dma_start(wm[:], d["wm"][:])
        nc.sync.dma_start(btau[:], d["btau"][:])
        nc.sync.dma_start(wgb[:], d["wg"][:])
        xbdma(0, nc.gpsimd)
        xdma(1)
        xdma(2)
        xdma(3, nc.gpsimd)
        xdma(4)
        xbdma(1, nc.gpsimd)
        xdma(5)
        xdma(6)
        xbdma(2, nc.gpsimd)
        xdma(7)
        xdma(8)
        xbdma(3, nc.gpsimd)
        xdma(9)
        xdma(10)
        xdma(11, nc.gpsimd)
        xdma(12)
        nc.sync.dma_start(
            womt[:].rearrange("p a b -> p (a b)"), d["wom"][:]
        )
        nc.sync.dma_start(i64[:], d["i64"][:])
        for i in range(2):
            sl = slice(i * 1600, (i + 1) * 1600)
            nc.sync.dma_start(xrr[:, sl], d["xr"][:, sl])

        with tc.tile_pool(name="fps", bufs=3, space="PSUM") as fps, \
             tc.tile_pool(name="yps", bufs=2, space="PSUM") as yps, \
             tc.tile_pool(name="esb", bufs=12) as esb, \
             tc.tile_pool(name="ep", bufs=4) as ep:

            def gt_batch(c0, nb, use_act):
                """Emits the conv matmuls; returns the drain for a later slot."""
                pgt = fps.tile([128, 2, 512], F32, tag="f")
                pg = pgt[:, 0, : nb * GT_W].rearrange(
                    "p (a b) -> p a b", b=GT_W)
                for i in range(nb):
                    nc.tensor.matmul(
                        pg[:, i, :],
                        lhsT=xb65[:, (c0 + i) * MC : (c0 + i + 1) * MC],
                        rhs=wgb[:],
                        start=(i == 0), stop=(i == nb - 1),
                    )

                def drain():
                    dst = gt[:, c0 : c0 + nb, :]
                    if use_act:
                        nc.scalar.activation(dst, pg[:, :nb, :], COPY)
                    else:
                        nc.vector.tensor_copy(dst, pg[:, :nb, :])
                return drain

            def th_slice(q0, w, psrc=None):
                if psrc is None:
                    pt = fps.tile([128, 2, 512], F32, tag="f")
                    pp = pt[0:C, 0, :]
                else:
                    pp = psrc
                nc.tensor.matmul(
                    pp[:, :w], lhsT=wm[:], rhs=xfr[:, q0 : q0 + w],
                    start=True, stop=True,
                )

                def drain():
                    nc.vector.tensor_scalar_add(
                        th[:, q0 : q0 + w], pp[:, :w], btau[:])
                return drain

            def make_epi_sched(pyf, py, subs, q0, w):
                """Slot-indexed task map for the NEXT block's pair slots.

                Consumers run 1-3 slots after their producers so every
                instruction enters its engine's in-order wait queue with
                dependencies already satisfied (no head-of-line blocking).
                """
                nsub = len(subs)
                holder = {}
                z = pyf[0:C, :]
                cnt = [0]

                def s_pys():
                    pys = ep.tile([128, 4, GT_W], F32, tag="pys")
                    nc.scalar.activation(
                        pys[:, :nsub, :], py[:, :nsub, :], COPY)
                    holder["pys"] = pys

                def s_recip():
                    pys = holder["pys"]
                    r = ep.tile([128, 4], F32, tag="r")
                    scr = ep.tile([128, 4], F32, tag="scr")
                    dview = pys[:, :, INTER : INTER + 1].rearrange(
                        "p a o -> p (a o)")
                    nc.vector.reciprocal_approx_accurate(
                        r[:, :nsub], dview[:, :nsub], scr[:, :nsub])
                    holder["r"] = r

                def mk_ynt(si):
                    def s():
                        ynt = ep.tile([128, INTER], BF16, tag="ynt")
                        nc.gpsimd.tensor_scalar(
                            ynt[:], holder["pys"][:, si, :INTER],
                            holder["r"][:, si : si + 1], None, op0=MULT,
                        )
                        holder[("ynt", si)] = ynt
                    return s

                def mk_bt(si):
                    def s():
                        bt = ep.tile([128, INTER], BF16, tag="bt")
                        nc.vector.transpose(bt[:], holder[("ynt", si)][:])
                        holder[("bt", si)] = bt
                    return s

                def mk_conv(si, soff):
                    def s():
                        for i in range(4):
                            bp = 32 * i
                            nc.tensor.matmul(
                                z[:, soff + bp : soff + bp + 32],
                                lhsT=womt[:, i, :],
                                rhs=holder[("bt", si)][:],
                                start=(cnt[0] == 0),
                                stop=False,
                            )
                            cnt[0] += 1
                    return s

                def s_resid():
                    nc.tensor.matmul(
                        z[:, :w], lhsT=i64[:], rhs=xrr[:, q0 : q0 + w],
                        start=False, stop=True,
                    )

                def s_out():
                    o = ep.tile([C, 512], F32, tag="o")
                    nc.scalar.activation(o[:, :w], z[:, :w], COPY)
                    nc.sync.dma_start(d["out"][:, q0 : q0 + w], o[:, :w])

                sched = {4: [s_pys], 5: [s_recip]}
                for si in range(nsub):
                    sched.setdefault(6 + si, []).append(mk_ynt(si))
                    sched.setdefault(8 + si, []).append(mk_bt(si))
                    sched.setdefault(9 + si, []).append(
                        mk_conv(si, subs[si]))
                sched.setdefault(10 + nsub, []).append(s_resid)
                sched.setdefault(12 + nsub, []).append(s_out)
                return sched

            # per-block fixed engine extras (ns) for the greedy exp split
            def extras(qi, w, nsub):
                ca_o = w * 0.8333 + 143.0          # o copy on ACT
                ca_pys = nsub * GT_W * 0.8333 + 143.0
                base_a = ca_o + ca_pys + 600.0     # fudge: measured ACT-heavy
                base_d = 65.0 + nsub * 94.0        # recip + bt transposes
                if qi + 1 < len(QBLOCKS):
                    base_d += QBLOCKS[qi + 1][1] * 1.0417 + 125.0  # th drain
                if qi == 0:
                    base_a += 4 * 405.0            # gt drains on ACT
                    base_d += 3 * 400.0 + 533.0 + 125.0   # gt + th0
                return base_a, base_d

            # y-flush queue is GLOBAL: a block's tail groups flush during the
            # next block's first slots so PE never bursts at a boundary.
            pending = []

            def flush_y():
                e, c, py, si, soff, nsub, first, last = pending.pop(0)
                nc.tensor.matmul(
                    py[:, si, :],
                    lhsT=e[:, c % 2, soff : soff + 128],
                    rhs=gt[:, c, :],
                    start=first, stop=last,
                )

            def push_pair(e, p, py, subs, nsub):
                for j in range(2):
                    c = 2 * p + j
                    for si, soff in enumerate(subs):
                        pending.append(
                            (e, c, py, si, soff, nsub,
                             c == 0 and si == 0,
                             c == NMC - 1 and si == nsub - 1))

            sched = {}
            prev_pyf = None
            nlast = len(QBLOCKS) - 1
            for qi, (q0, w) in enumerate(QBLOCKS):
                if qi == 0:
                    th_slice(q0, w)()   # nothing to overlap with yet
                subs = list(range(0, w, 128))
                nsub = len(subs)
                pyf = yps.tile([128, 512], F32, tag="py")
                py = pyf[:, : 4 * GT_W].rearrange("p (a b) -> p a b", b=GT_W)

                base_a, base_d = extras(qi, w, nsub)
                engs = _mk_engs(NPAIR, w, base_a, base_d)
                gtd = [True, False, True, False, True, False, True]
                # groups (y matmuls) to keep in flight; one pair = 2*nsub
                lag = (1 if qi == nlast else 3) * 2 * nsub

                if qi == 0:
                    sched.setdefault(0, []).append(gt_batch(0, 8, gtd[0]))
                for p in range(NPAIR):
                    pf = fps.tile([128, 2, 512], F32, tag="f")
                    for j in range(2):
                        c = 2 * p + j
                        nc.tensor.matmul(
                            pf[:, j, :w],
                            lhsT=xfr[:, c * MC : (c + 1) * MC],
                            rhs=th[:, q0 : q0 + w],
                            start=True, stop=True,
                        )
                    if qi == 0 and p % 4 == 1 and p < 22:
                        k = (p - 1) // 4
                        c0 = 8 * k + 8
                        sched.setdefault(p + 1, []).append(
                            gt_batch(c0, min(8, NMC - c0), gtd[k + 1]))
                    nfl = 0
                    while len(pending) > lag and nfl < 2 * nsub + 4:
                        flush_y()
                        nfl += 1
                    for fn in sched.pop(p, []):
                        fn()
                    if qi + 1 < len(QBLOCKS):
                        nq0, nw = QBLOCKS[qi + 1]
                        if p == 20:
                            sched.setdefault(21, []).append(th_slice(
                                nq0, nw,
                                psrc=(prev_pyf[0:C, :nw]
                                      if prev_pyf is not None else None)))
                    e = esb.tile([128, 2, 512], BF16, tag="e")
                    if engs[p]:
                        nc.scalar.activation(e[:, :, :w], pf[:, :, :w], EXP)
                    else:
                        nc.vector.tensor_scalar(
                            e[:, :, :w].bitcast(I16), pf[:, :, :w],
                            A_EXP, B_EXP, op0=MULT, op1=ADD,
                        )
                    push_pair(e, p, py, subs, nsub)
                prev_pyf = pyf
                assert not sched, f"unconsumed epilogue slots: {sorted(sched)}"
                sched = make_epi_sched(pyf, py, subs, q0, w)
            while pending:
                flush_y()
            for slot in sorted(sched):
                for fn in sched[slot]:
                    fn()


def build():
    nc = bacc.Bacc("TRN2", target_bir_lowering=False, debug=False)
    d = {}
    d["xf"] = nc.dram_tensor("xf", [C, N], F32R, kind="ExternalInput").ap()
    d["xb"] = nc.dram_tensor("xb", [C + 1, N], BF16, kind="ExternalInput").ap()
    d["xr"] = nc.dram_tensor("xr", [C, NQ], F32R, kind="ExternalInput").ap()
    d["wm"] = nc.dram_tensor("wm", [C, C], F32R, kind="ExternalInput").ap()
    d["btau"] = nc.dram_tensor("btau", [C, 1], F32, kind="ExternalInput").ap()
    d["wg"] = nc.dram_tensor("wg", [C + 1, GT_W], BF16,
                             kind="ExternalInput").ap()
    d["wom"] = nc.dram_tensor("wom", [128, 4 * C], BF16,
                              kind="ExternalInput").ap()
    d["i64"] = nc.dram_tensor("i64", [C, C], F32R, kind="ExternalInput").ap()
    d["out"] = nc.dram_tensor("out", [C, NQ], F32, kind="ExternalOutput").ap()
    with tile.TileContext(nc) as tc:
        _emit(tc, d)
    nc.compile()
    return nc


def make_in_maps(x, w_theta, b_theta, w_phi, b_phi, w_g, b_g,
                 w_out, b_out, bn_gamma, bn_beta, bn_mean, bn_var):
    x = np.ascontiguousarray(np.asarray(x, dtype=np.float32))
    w_theta = np.asarray(w_theta, np.float32)
    b_theta = np.asarray(b_theta, np.float32)
    w_phi = np.asarray(w_phi, np.float32)
    w_g = np.asarray(w_g, np.float32)
    b_g = np.asarray(b_g, np.float32)
    w_out = np.asarray(w_out, np.float32)
    b_out = np.asarray(b_out, np.float32)
    bn_gamma = np.asarray(bn_gamma, np.float32)
    bn_beta = np.asarray(bn_beta, np.float32)
    bn_mean = np.asarray(bn_mean, np.float32)
    bn_var = np.asarray(bn_var, np.float32)

    inv = bn_gamma / np.sqrt(bn_var + BN_EPS)
    wo_folded = w_out * inv[:, None]                       # [64,32]
    bo_folded = (b_out - bn_mean) * inv + bn_beta          # [64]

    # f[q,k] = x_q.T M x_k + (Wph.T b_theta).x_q-side bias carried by theta;
    # per-query terms are softmax-row-invariant and dropped.
    wm_l = np.ascontiguousarray(w_theta.T @ w_phi)         # [64,64]
    btau_c = np.ascontiguousarray((w_phi.T @ b_theta)[:, None])  # [64,1]
    wg65 = np.zeros((C + 1, GT_W), np.float32)
    wg65[:C, :INTER] = w_g.T
    wg65[C, :INTER] = b_g
    wg65[C, INTER] = 1.0                                   # denominator ones
    wg65 = np.ascontiguousarray(wg65.astype(ml_dtypes.bfloat16))
    wom = np.zeros((128, 4, C), np.float32)
    for i in range(4):
        wom[32 * i : 32 * i + 32, i, :] = wo_folded.T
    wom = np.ascontiguousarray(
        wom.reshape(128, 4 * C).astype(ml_dtypes.bfloat16))
    i64 = np.eye(C, dtype=np.float32)

    xflat = x.reshape(B, C, N)
    in_maps = []
    for core in range(NCORES):
        b, h = divmod(core, 2)
        xrot = np.ascontiguousarray(np.roll(xflat[b], -h * NQ, axis=1))
        xb65 = np.ones((C + 1, N), np.float32)
        xb65[:C] = xrot
        xb65 = np.ascontiguousarray(xb65.astype(ml_dtypes.bfloat16))
        xres = np.ascontiguousarray(xrot[:, :NQ] + bo_folded[:, None])
        in_maps.append(
            {
                "xf": xrot,
                "xb": xb65,
                "xr": xres,
                "wm": wm_l,
                "btau": btau_c,
                "wg": wg65,
                "wom": wom,
                "i64": i64,
            }
        )
    return in_maps


def assemble_out(results):
    out = np.empty((B, C, N), np.float32)
    for core in range(NCORES):
        b, h = divmod(core, 2)
        out[b][:, h * NQ : (h + 1) * NQ] = results[core]["out"]
    return out.reshape(B, C, HH, WW)


_NC_CACHE = [None]


def kernel(**inputs):
    if _NC_CACHE[0] is None:
        _NC_CACHE[0] = build()
    nc = _NC_CACHE[0]
    in_maps = make_in_maps(**inputs)
    res = run_bass_kernel_spmd(nc, in_maps, core_ids=list(range(NCORES)))
    return assemble_out(res.results)
